# revision 10
# baseline (speedup 1.0000x reference)
"""Trainium2 Bass kernel for nn_DDNWithResidualLoss.

Contract: kernel(**inputs) takes the FULL unsharded inputs (numpy arrays,
keyed as in reference.setup_inputs()) and returns the FULL output (the two
scalar losses). The batch dim B=8 is sharded 1 image per NeuronCore across
8 cores; the box list shards with its image; per-core partial weighted sums
are combined on the host (the cross-device psum is 16 floats).

Design: the only O(C*H*W) device work the loss needs is the softmax
denominator s[px] = sum_c exp(x[c,px]). The target-bin gathers (x_t, r_t)
depend only on the host-computed box rasterization, so the host gathers
them per pixel (O(H*W), same spirit as the baseline's host-gathered
residual candidate rows) and ships three small per-pixel planes. The
device then streams the logits in a pixel-major [128, 240*81] fp16 layout
(full 128-partition DMA + ACT width), exps on the Activation engine
(1 elem/cycle/lane, the 16.2us floor), segment-reduces 81 channels per
pixel on DVE in fp16 (2x/4x mode), and runs a short per-pixel epilogue:

  q = x_t - ln s;  p = exp(q);  lnp = ln(p + 1e-8);  u = 1 - p
  loss_map_px = A*u^2*(-lnp),  loss_res_px = Ad*u^2
  with A = w*alpha and Ad = A*|r_t - res_target| host-folded.

Everything is fp16 on the wire (validated: rel err ~2e-6 vs reference).
A dummy activation at t=0 hides the ~2.7us act-table load under the
first DMA block; exp and ln share one table set (natural_log_exp).
"""

import numpy as np

# ---------------- problem constants (hardcoded per contract) ----------------
B, D, H, W = 8, 80, 96, 320
C = D + 1              # 81 channels
HW = H * W             # 30720 pixels
P = 128                # SBUF partitions
KPP = HW // P          # 240 pixels per partition row
FREE = KPP * C         # 19440 fp16 elements per partition
ALPHA, GAMMA = 0.25, 2.0
FG_W, BG_W = 13.0, 1.0
DEPTH_MIN, DEPTH_MAX = 0.001, 60.0
EPS = 1e-8
N_CORES = 8

# streaming block sizes in pixel-columns. Uniform 30-px blocks measured
# fastest: a ramped schedule (8,16,32,...) regressed 30.1us -> 35.3us (the
# extra DMA/ACT instruction rounds cost more than the fill/tail they saved).
BLK_SIZES = (30,) * 8
assert sum(BLK_SIZES) == KPP

f32 = np.float32
f16 = np.float16


# ---------------- host-side reference-exact target computation ----------------
def _host_targets(gt_boxes2d, num_gt_per_img, gt_center_depth):
    """Bit-exact float32 replication of the reference's rasterization+binning.

    Returns per-pixel planes (B, H, W): depth bin target (int32),
    residual target (f32), balancer weight (f32).
    """
    gt_boxes2d = np.asarray(gt_boxes2d, f32)
    gt_center_depth = np.asarray(gt_center_depth, f32)
    num_gt = np.asarray(num_gt_per_img, np.int64)

    u1 = np.floor(gt_boxes2d[:, 0]).astype(np.int32)
    v1 = np.floor(gt_boxes2d[:, 1]).astype(np.int32)
    u2 = np.ceil(gt_boxes2d[:, 2]).astype(np.int32)
    v2 = np.ceil(gt_boxes2d[:, 3]).astype(np.int32)
    ntot = gt_boxes2d.shape[0]

    # jnp.repeat(..., total_repeat_length=ntot): truncate, or pad with the
    # final value (matches jax semantics for the padded tail).
    rep = np.repeat(np.arange(B), np.clip(num_gt, 0, None))
    if len(rep) >= ntot:
        rep = rep[:ntot]
    else:
        pad_val = rep[-1] if len(rep) else 0
        rep = np.concatenate([rep, np.full(ntot - len(rep), pad_val, rep.dtype)])

    dm = np.full((B, H, W), DEPTH_MAX, f32)
    fg = np.zeros((B, H, W), bool)
    for i in range(ntot):
        b = int(rep[i])
        ys = slice(max(int(v1[i]), 0), max(int(v2[i]), 0))
        xs = slice(max(int(u1[i]), 0), max(int(u2[i]), 0))
        dm[b, ys, xs] = np.minimum(dm[b, ys, xs], gt_center_depth[i])
        fg[b, ys, xs] = True

    num_bins = D
    bin_size = f32(2.0 * (DEPTH_MAX - DEPTH_MIN) / (num_bins * (1 + num_bins)))
    with np.errstate(invalid="ignore"):
        idx = f32(-0.5) + f32(0.5) * np.sqrt(
            f32(1.0) + f32(8.0) * (dm - f32(DEPTH_MIN)) / bin_size, dtype=f32
        )
        bad = (idx < 0) | (idx > num_bins) | ~np.isfinite(idx)
        tgt = np.where(bad, num_bins, np.floor(np.where(bad, 0, idx))).astype(np.int32)

    bi = np.arange(num_bins, dtype=f32)
    bin_value = (bi + f32(0.5)) ** 2 * bin_size / f32(2.0) - bin_size / f32(8.0) + f32(DEPTH_MIN)
    bin_values = np.concatenate([bin_value, np.array([DEPTH_MAX], f32)])

    res_tgt = (dm - bin_values[tgt]).astype(f32)
    wgt = np.where(fg, f32(FG_W), f32(BG_W))
    return tgt, res_tgt, wgt


# ---------------- device program ----------------
_PROGRAM = None


def _build_program(loop_iters=None):
    """Build the SPMD program. loop_iters (benchmark only): wrap the body in
    an on-device For loop so one NEFF executes the kernel body N times,
    letting wall-clock measurements amortize launch/transfer overhead."""
    import concourse.tile as tile
    from concourse import bacc, mybir
    from contextlib import ExitStack, nullcontext

    dt = mybir.dt
    Alu = mybir.AluOpType
    Act = mybir.ActivationFunctionType

    nc = bacc.Bacc("TRN2", target_bir_lowering=False, debug=False)

    x_d = nc.declare_dram_parameter("x16", [P, FREE], dt.float16, isOutput=False)
    xt_d = nc.declare_dram_parameter("xt", [P, KPP], dt.float32, isOutput=False)
    a_d = nc.declare_dram_parameter("aw", [P, KPP], dt.float32, isOutput=False)
    ad_d = nc.declare_dram_parameter("adw", [P, KPP], dt.float32, isOutput=False)
    out_d = nc.declare_dram_parameter("out", [P, 2], dt.float32, isOutput=True)

    with tile.TileContext(nc) as tc, ExitStack() as ctx:
        const_p = ctx.enter_context(tc.tile_pool(name="const", bufs=1))
        stage_p = ctx.enter_context(tc.tile_pool(name="stage", bufs=2))
        small_p = ctx.enter_context(tc.tile_pool(name="small", bufs=1))

        # warm the exp/ln activation table at t=0 so the ~2.7us table load
        # overlaps the first DMA block instead of serializing after it
        warm = const_p.tile([P, 1], dt.float32)
        nc.gpsimd.memset(warm[:], 0.0)
        nc.scalar.activation(warm[:], warm[:], Act.Exp)

        eps_t = const_p.tile([P, 1], dt.float32)
        nc.gpsimd.memset(eps_t[:], EPS)

        xt_t = const_p.tile([P, KPP], dt.float32)
        a_t = const_p.tile([P, KPP], dt.float32)
        ad_t = const_p.tile([P, KPP], dt.float32)

        def load_planes():
            nc.sync.dma_start(out=xt_t[:], in_=xt_d[:])
            nc.sync.dma_start(out=a_t[:], in_=a_d[:])
            nc.sync.dma_start(out=ad_t[:], in_=ad_d[:])

        if loop_iters:
            # loop-invariant: load once, before entering the bench loop
            load_planes()

        loop_cm = (tc.For_i(0, loop_iters, 1, hint_engines=(nc.scalar.engine,))
                   if loop_iters else nullcontext())
        ctx.enter_context(loop_cm)

        s_t = small_p.tile([P, KPP], dt.float16)

        off = 0
        for kpb in BLK_SIZES:
            cpb = kpb * C
            xs = stage_p.tile([P, cpb], dt.float16, tag=f"xs{kpb}")
            nc.sync.dma_start(out=xs[:], in_=x_d[:, off * C:off * C + cpb])
            es = stage_p.tile([P, cpb], dt.float16, tag=f"es{kpb}")
            nc.scalar.activation(es[:], xs[:], Act.Exp)
            esv = es[:].rearrange("p (k c) -> p k c", c=C)
            # fp16 accumulation over 81 terms: validated rel err ~2e-6 vs
            # reference (tolerance 2e-2); fp16 out keeps the DVE 2x mode
            with nc.allow_low_precision(reason="81-term softmax denom; validated 2e-6"):
                nc.vector.tensor_reduce(s_t[:, off:off + kpb], esv,
                                        axis=mybir.AxisListType.X, op=Alu.add)
            off += kpb

        if not loop_iters:
            # single-shot: planes aren't read until the epilogue (~21us in),
            # so load them after the x stream — they'd otherwise sit in front
            # of block 0 on the SP queue and delay the first exp
            load_planes()

        # ---- per-pixel loss epilogue on [128, 240] planes ----
        # minimal dependent-hop chain:
        #   ACT: ln_s -> DVE: q -> ACT: p, lnp  ||  DVE: u, f -> af -> ttr1, ttr2
        ln_s = small_p.tile([P, KPP], dt.float32)
        nc.scalar.activation(ln_s[:], s_t[:], Act.Ln)
        q = small_p.tile([P, KPP], dt.float32)
        nc.vector.tensor_sub(q[:], xt_t[:], ln_s[:])
        p_t = small_p.tile([P, KPP], dt.float32)
        nc.scalar.activation(p_t[:], q[:], Act.Exp)
        lnp = small_p.tile([P, KPP], dt.float32)
        nc.scalar.activation(lnp[:], p_t[:], Act.Ln, bias=eps_t[:])
        u = small_p.tile([P, KPP], dt.float32)
        nc.vector.tensor_scalar(u[:], p_t[:], -1.0, 1.0, op0=Alu.mult, op1=Alu.add)
        f_t = small_p.tile([P, KPP], dt.float32)
        nc.vector.tensor_mul(f_t[:], u[:], u[:])
        af = small_p.tile([P, KPP], dt.float32)
        nc.vector.tensor_mul(af[:], f_t[:], a_t[:])

        part = small_p.tile([P, 2], dt.float32)
        h1 = small_p.tile([P, KPP], dt.float32)
        nc.vector.tensor_mul(h1[:], af[:], lnp[:])
        nc.vector.tensor_reduce(part[:, 0:1], h1[:], axis=mybir.AxisListType.X,
                                op=Alu.add)
        h2 = small_p.tile([P, KPP], dt.float32)
        nc.vector.tensor_mul(h2[:], f_t[:], ad_t[:])
        nc.vector.tensor_reduce(part[:, 1:2], h2[:], axis=mybir.AxisListType.X,
                                op=Alu.add)
        nc.sync.dma_start(out=out_d[:], in_=part[:])

    nc.compile()
    return nc


def _get_program():
    global _PROGRAM
    if _PROGRAM is None:
        _PROGRAM = _build_program()
    return _PROGRAM


LAST_RESULTS = None  # populated with the BassKernelResults of the last run


def _build_in_maps(depth_logits, depth_residuals, tgt, res_tgt, wgt):
    """depth_logits/depth_residuals: (B, C, HW); tgt/res_tgt/wgt: (B, ...)"""
    pix = np.arange(HW)
    in_maps = []
    for b in range(N_CORES):
        x = depth_logits[b]                       # (C, HW) f32
        t = tgt[b].reshape(HW)
        xt = x[t, pix]                            # target-bin logit per pixel
        rp = depth_residuals[b][t, pix]           # target-bin residual per pixel
        dres = np.abs(rp - res_tgt[b].reshape(HW))
        aw = (wgt[b].reshape(HW) * f32(ALPHA)).astype(f32)
        adw = (aw * dres).astype(f32)
        # pixel-major fp16 layout: partition p holds pixels [p*240, (p+1)*240),
        # each pixel's 81 channel values contiguous
        xp = np.ascontiguousarray(x.T).astype(f16).reshape(P, FREE)
        in_maps.append({
            "x16": xp,
            "xt": np.ascontiguousarray(xt.reshape(P, KPP)),
            "aw": np.ascontiguousarray(aw.reshape(P, KPP)),
            "adw": np.ascontiguousarray(adw.reshape(P, KPP)),
        })
    return in_maps


def kernel(depth_logits, depth_residuals, gt_boxes2d, num_gt_per_img, gt_center_depth):
    global LAST_RESULTS
    from concourse.bass_utils import run_bass_kernel_spmd

    depth_logits = np.asarray(depth_logits, f32).reshape(B, C, HW)
    depth_residuals = np.asarray(depth_residuals, f32).reshape(B, C, HW)

    tgt, res_tgt, wgt = _host_targets(gt_boxes2d, num_gt_per_img, gt_center_depth)
    in_maps = _build_in_maps(depth_logits, depth_residuals, tgt, res_tgt, wgt)

    nc = _get_program()
    res = run_bass_kernel_spmd(nc, in_maps, list(range(N_CORES)))
    LAST_RESULTS = res

    acc = np.zeros(2, np.float64)
    for b in range(N_CORES):
        acc += np.asarray(res.results[b]["out"], np.float64).sum(axis=0)
    num_pixels = float(B * H * W)
    map_loss = f32(-acc[0] / num_pixels)
    res_loss = f32(acc[1] / num_pixels)
    return map_loss, res_loss


# revision 11
# speedup vs baseline: 1.2274x; 1.2274x over previous
"""Trainium2 Bass kernel for nn_DDNWithResidualLoss.

Contract: kernel(**inputs) takes the FULL unsharded inputs (numpy arrays,
keyed as in reference.setup_inputs()) and returns the FULL output (the two
scalar losses). The batch dim B=8 is sharded 1 image per NeuronCore across
8 cores; the box list shards with its image; per-core partial weighted sums
are combined on the host (the cross-device psum is 16 floats).

Design: the only O(C*H*W) device work the loss needs is the softmax
denominator s[px] = sum_c exp(x[c,px]). The target-bin gathers (x_t, r_t)
depend only on the host-computed box rasterization, so the host gathers
them per pixel (O(H*W), same spirit as the baseline's host-gathered
residual candidate rows) and ships three small per-pixel planes. The
device then streams the logits in a pixel-major [128, 240*81] fp16 layout
(full 128-partition DMA + ACT width), exps on the Activation engine
(1 elem/cycle/lane, the 16.2us floor), segment-reduces 81 channels per
pixel on DVE in fp16 (2x/4x mode), and runs a short per-pixel epilogue:

  q = x_t - ln s;  p = exp(q);  lnp = ln(p + 1e-8);  u = 1 - p
  loss_map_px = A*u^2*(-lnp),  loss_res_px = Ad*u^2
  with A = w*alpha and Ad = A*|r_t - res_target| host-folded.

Everything is fp16 on the wire (validated: rel err ~2e-6 vs reference).
A dummy activation at t=0 hides the ~2.7us act-table load under the
first DMA block; exp and ln share one table set (natural_log_exp).
"""

import numpy as np

# ---------------- problem constants (hardcoded per contract) ----------------
B, D, H, W = 8, 80, 96, 320
C = D + 1              # 81 channels
HW = H * W             # 30720 pixels
P = 128                # SBUF partitions
KPP = HW // P          # 240 pixels per partition row
FREE = KPP * C         # 19440 fp16 elements per partition
ALPHA, GAMMA = 0.25, 2.0
FG_W, BG_W = 13.0, 1.0
DEPTH_MIN, DEPTH_MAX = 0.001, 60.0
EPS = 1e-8
N_CORES = 8

# streaming block sizes in pixel-columns: small first blocks shorten the
# DMA pipeline fill before the first exp; small last blocks shorten the
# last-exp -> last-reduce -> epilogue serial tail. A same-session
# alternating A/B measured this ramp ~22% faster than uniform (30,)*8
# (cross-session wall-clock comparisons proved unreliable, +/-20us drift).
BLK_SIZES = (8, 16, 32, 32, 32, 32, 32, 32, 16, 8)
assert sum(BLK_SIZES) == KPP

f32 = np.float32
f16 = np.float16


# ---------------- host-side reference-exact target computation ----------------
def _host_targets(gt_boxes2d, num_gt_per_img, gt_center_depth):
    """Bit-exact float32 replication of the reference's rasterization+binning.

    Returns per-pixel planes (B, H, W): depth bin target (int32),
    residual target (f32), balancer weight (f32).
    """
    gt_boxes2d = np.asarray(gt_boxes2d, f32)
    gt_center_depth = np.asarray(gt_center_depth, f32)
    num_gt = np.asarray(num_gt_per_img, np.int64)

    u1 = np.floor(gt_boxes2d[:, 0]).astype(np.int32)
    v1 = np.floor(gt_boxes2d[:, 1]).astype(np.int32)
    u2 = np.ceil(gt_boxes2d[:, 2]).astype(np.int32)
    v2 = np.ceil(gt_boxes2d[:, 3]).astype(np.int32)
    ntot = gt_boxes2d.shape[0]

    # jnp.repeat(..., total_repeat_length=ntot): truncate, or pad with the
    # final value (matches jax semantics for the padded tail).
    rep = np.repeat(np.arange(B), np.clip(num_gt, 0, None))
    if len(rep) >= ntot:
        rep = rep[:ntot]
    else:
        pad_val = rep[-1] if len(rep) else 0
        rep = np.concatenate([rep, np.full(ntot - len(rep), pad_val, rep.dtype)])

    dm = np.full((B, H, W), DEPTH_MAX, f32)
    fg = np.zeros((B, H, W), bool)
    for i in range(ntot):
        b = int(rep[i])
        ys = slice(max(int(v1[i]), 0), max(int(v2[i]), 0))
        xs = slice(max(int(u1[i]), 0), max(int(u2[i]), 0))
        dm[b, ys, xs] = np.minimum(dm[b, ys, xs], gt_center_depth[i])
        fg[b, ys, xs] = True

    num_bins = D
    bin_size = f32(2.0 * (DEPTH_MAX - DEPTH_MIN) / (num_bins * (1 + num_bins)))
    with np.errstate(invalid="ignore"):
        idx = f32(-0.5) + f32(0.5) * np.sqrt(
            f32(1.0) + f32(8.0) * (dm - f32(DEPTH_MIN)) / bin_size, dtype=f32
        )
        bad = (idx < 0) | (idx > num_bins) | ~np.isfinite(idx)
        tgt = np.where(bad, num_bins, np.floor(np.where(bad, 0, idx))).astype(np.int32)

    bi = np.arange(num_bins, dtype=f32)
    bin_value = (bi + f32(0.5)) ** 2 * bin_size / f32(2.0) - bin_size / f32(8.0) + f32(DEPTH_MIN)
    bin_values = np.concatenate([bin_value, np.array([DEPTH_MAX], f32)])

    res_tgt = (dm - bin_values[tgt]).astype(f32)
    wgt = np.where(fg, f32(FG_W), f32(BG_W))
    return tgt, res_tgt, wgt


# ---------------- device program ----------------
_PROGRAM = None


def _build_program(loop_iters=None):
    """Build the SPMD program. loop_iters (benchmark only): wrap the body in
    an on-device For loop so one NEFF executes the kernel body N times,
    letting wall-clock measurements amortize launch/transfer overhead."""
    import concourse.tile as tile
    from concourse import bacc, mybir
    from contextlib import ExitStack, nullcontext

    dt = mybir.dt
    Alu = mybir.AluOpType
    Act = mybir.ActivationFunctionType

    nc = bacc.Bacc("TRN2", target_bir_lowering=False, debug=False)

    x_d = nc.declare_dram_parameter("x16", [P, FREE], dt.float16, isOutput=False)
    xt_d = nc.declare_dram_parameter("xt", [P, KPP], dt.float32, isOutput=False)
    a_d = nc.declare_dram_parameter("aw", [P, KPP], dt.float32, isOutput=False)
    ad_d = nc.declare_dram_parameter("adw", [P, KPP], dt.float32, isOutput=False)
    out_d = nc.declare_dram_parameter("out", [P, 2], dt.float32, isOutput=True)

    with tile.TileContext(nc) as tc, ExitStack() as ctx:
        const_p = ctx.enter_context(tc.tile_pool(name="const", bufs=1))
        stage_p = ctx.enter_context(tc.tile_pool(name="stage", bufs=2))
        small_p = ctx.enter_context(tc.tile_pool(name="small", bufs=1))

        # warm the exp/ln activation table at t=0 so the ~2.7us table load
        # overlaps the first DMA block instead of serializing after it
        warm = const_p.tile([P, 1], dt.float32)
        nc.gpsimd.memset(warm[:], 0.0)
        nc.scalar.activation(warm[:], warm[:], Act.Exp)

        eps_t = const_p.tile([P, 1], dt.float32)
        nc.gpsimd.memset(eps_t[:], EPS)

        xt_t = const_p.tile([P, KPP], dt.float32)
        a_t = const_p.tile([P, KPP], dt.float32)
        ad_t = const_p.tile([P, KPP], dt.float32)

        def load_planes():
            nc.sync.dma_start(out=xt_t[:], in_=xt_d[:])
            nc.sync.dma_start(out=a_t[:], in_=a_d[:])
            nc.sync.dma_start(out=ad_t[:], in_=ad_d[:])

        if loop_iters:
            # loop-invariant: load once, before entering the bench loop
            load_planes()

        loop_cm = (tc.For_i(0, loop_iters, 1, hint_engines=(nc.scalar.engine,))
                   if loop_iters else nullcontext())
        ctx.enter_context(loop_cm)

        s_t = small_p.tile([P, KPP], dt.float16)

        off = 0
        for kpb in BLK_SIZES:
            cpb = kpb * C
            xs = stage_p.tile([P, cpb], dt.float16, tag=f"xs{kpb}")
            nc.sync.dma_start(out=xs[:], in_=x_d[:, off * C:off * C + cpb])
            es = stage_p.tile([P, cpb], dt.float16, tag=f"es{kpb}")
            nc.scalar.activation(es[:], xs[:], Act.Exp)
            esv = es[:].rearrange("p (k c) -> p k c", c=C)
            # fp16 accumulation over 81 terms: validated rel err ~2e-6 vs
            # reference (tolerance 2e-2); fp16 out keeps the DVE 2x mode
            with nc.allow_low_precision(reason="81-term softmax denom; validated 2e-6"):
                nc.vector.tensor_reduce(s_t[:, off:off + kpb], esv,
                                        axis=mybir.AxisListType.X, op=Alu.add)
            off += kpb

        if not loop_iters:
            # single-shot: planes aren't read until the epilogue (~21us in),
            # so load them after the x stream — they'd otherwise sit in front
            # of block 0 on the SP queue and delay the first exp
            load_planes()

        # ---- per-pixel loss epilogue on [128, 240] planes ----
        # minimal dependent-hop chain:
        #   ACT: ln_s -> DVE: q -> ACT: p, lnp  ||  DVE: u, f -> af -> ttr1, ttr2
        ln_s = small_p.tile([P, KPP], dt.float32)
        nc.scalar.activation(ln_s[:], s_t[:], Act.Ln)
        q = small_p.tile([P, KPP], dt.float32)
        nc.vector.tensor_sub(q[:], xt_t[:], ln_s[:])
        p_t = small_p.tile([P, KPP], dt.float32)
        nc.scalar.activation(p_t[:], q[:], Act.Exp)
        lnp = small_p.tile([P, KPP], dt.float32)
        nc.scalar.activation(lnp[:], p_t[:], Act.Ln, bias=eps_t[:])
        u = small_p.tile([P, KPP], dt.float32)
        nc.vector.tensor_scalar(u[:], p_t[:], -1.0, 1.0, op0=Alu.mult, op1=Alu.add)
        f_t = small_p.tile([P, KPP], dt.float32)
        nc.vector.tensor_mul(f_t[:], u[:], u[:])
        af = small_p.tile([P, KPP], dt.float32)
        nc.vector.tensor_mul(af[:], f_t[:], a_t[:])

        part = small_p.tile([P, 2], dt.float32)
        h1 = small_p.tile([P, KPP], dt.float32)
        nc.vector.tensor_mul(h1[:], af[:], lnp[:])
        nc.vector.tensor_reduce(part[:, 0:1], h1[:], axis=mybir.AxisListType.X,
                                op=Alu.add)
        h2 = small_p.tile([P, KPP], dt.float32)
        nc.vector.tensor_mul(h2[:], f_t[:], ad_t[:])
        nc.vector.tensor_reduce(part[:, 1:2], h2[:], axis=mybir.AxisListType.X,
                                op=Alu.add)
        nc.sync.dma_start(out=out_d[:], in_=part[:])

    nc.compile()
    return nc


def _get_program():
    global _PROGRAM
    if _PROGRAM is None:
        _PROGRAM = _build_program()
    return _PROGRAM


LAST_RESULTS = None  # populated with the BassKernelResults of the last run


def _build_in_maps(depth_logits, depth_residuals, tgt, res_tgt, wgt):
    """depth_logits/depth_residuals: (B, C, HW); tgt/res_tgt/wgt: (B, ...)"""
    pix = np.arange(HW)
    in_maps = []
    for b in range(N_CORES):
        x = depth_logits[b]                       # (C, HW) f32
        t = tgt[b].reshape(HW)
        xt = x[t, pix]                            # target-bin logit per pixel
        rp = depth_residuals[b][t, pix]           # target-bin residual per pixel
        dres = np.abs(rp - res_tgt[b].reshape(HW))
        aw = (wgt[b].reshape(HW) * f32(ALPHA)).astype(f32)
        adw = (aw * dres).astype(f32)
        # pixel-major fp16 layout: partition p holds pixels [p*240, (p+1)*240),
        # each pixel's 81 channel values contiguous
        xp = np.ascontiguousarray(x.T).astype(f16).reshape(P, FREE)
        in_maps.append({
            "x16": xp,
            "xt": np.ascontiguousarray(xt.reshape(P, KPP)),
            "aw": np.ascontiguousarray(aw.reshape(P, KPP)),
            "adw": np.ascontiguousarray(adw.reshape(P, KPP)),
        })
    return in_maps


def kernel(depth_logits, depth_residuals, gt_boxes2d, num_gt_per_img, gt_center_depth):
    global LAST_RESULTS
    from concourse.bass_utils import run_bass_kernel_spmd

    depth_logits = np.asarray(depth_logits, f32).reshape(B, C, HW)
    depth_residuals = np.asarray(depth_residuals, f32).reshape(B, C, HW)

    tgt, res_tgt, wgt = _host_targets(gt_boxes2d, num_gt_per_img, gt_center_depth)
    in_maps = _build_in_maps(depth_logits, depth_residuals, tgt, res_tgt, wgt)

    nc = _get_program()
    res = run_bass_kernel_spmd(nc, in_maps, list(range(N_CORES)))
    LAST_RESULTS = res

    acc = np.zeros(2, np.float64)
    for b in range(N_CORES):
        acc += np.asarray(res.results[b]["out"], np.float64).sum(axis=0)
    num_pixels = float(B * H * W)
    map_loss = f32(-acc[0] / num_pixels)
    res_loss = f32(acc[1] / num_pixels)
    return map_loss, res_loss


# revision 13
# speedup vs baseline: 1.2550x; 1.0225x over previous
"""Trainium2 Bass kernel for nn_DDNWithResidualLoss.

Contract: kernel(**inputs) takes the FULL unsharded inputs (numpy arrays,
keyed as in reference.setup_inputs()) and returns the FULL output (the two
scalar losses). The batch dim B=8 is sharded 1 image per NeuronCore across
8 cores; the box list shards with its image; per-core partial weighted sums
are combined on the host (the cross-device psum is 16 floats).

Design: the only O(C*H*W) device work the loss needs is the softmax
denominator s[px] = sum_c exp(x[c,px]). The target-bin gathers (x_t, r_t)
depend only on the host-computed box rasterization, so the host gathers
them per pixel (O(H*W), same spirit as the baseline's host-gathered
residual candidate rows) and ships three small per-pixel planes. The
device then streams the logits in a pixel-major [128, 240*81] fp16 layout
(full 128-partition DMA + ACT width), exps on the Activation engine
(1 elem/cycle/lane, the 16.2us floor), segment-reduces 81 channels per
pixel on DVE in fp16 (2x/4x mode), and runs a short per-pixel epilogue:

  q = x_t - ln s;  p = exp(q);  lnp = ln(p + 1e-8);  u = 1 - p
  loss_map_px = A*u^2*(-lnp),  loss_res_px = Ad*u^2
  with A = w*alpha and Ad = A*|r_t - res_target| host-folded.

Everything is fp16 on the wire (validated: rel err ~2e-6 vs reference).
A dummy activation at t=0 hides the ~2.7us act-table load under the
first DMA block; exp and ln share one table set (natural_log_exp).
"""

import numpy as np

# ---------------- problem constants (hardcoded per contract) ----------------
B, D, H, W = 8, 80, 96, 320
C = D + 1              # 81 channels
HW = H * W             # 30720 pixels
P = 128                # SBUF partitions
KPP = HW // P          # 240 pixels per partition row
FREE = KPP * C         # 19440 fp16 elements per partition
ALPHA, GAMMA = 0.25, 2.0
FG_W, BG_W = 13.0, 1.0
DEPTH_MIN, DEPTH_MAX = 0.001, 60.0
EPS = 1e-8
N_CORES = 8

# streaming block sizes in pixel-columns: small first blocks shorten the
# DMA pipeline fill before the first exp; small last blocks shorten the
# last-exp -> last-reduce -> epilogue serial tail. A same-session
# alternating A/B measured this ramp ~22% faster than uniform (30,)*8
# (cross-session wall-clock comparisons proved unreliable, +/-20us drift).
BLK_SIZES = (8, 16, 32, 32, 32, 32, 32, 32, 16, 8)
assert sum(BLK_SIZES) == KPP

f32 = np.float32
f16 = np.float16


# ---------------- host-side reference-exact target computation ----------------
def _host_targets(gt_boxes2d, num_gt_per_img, gt_center_depth):
    """Bit-exact float32 replication of the reference's rasterization+binning.

    Returns per-pixel planes (B, H, W): depth bin target (int32),
    residual target (f32), balancer weight (f32).
    """
    gt_boxes2d = np.asarray(gt_boxes2d, f32)
    gt_center_depth = np.asarray(gt_center_depth, f32)
    num_gt = np.asarray(num_gt_per_img, np.int64)

    u1 = np.floor(gt_boxes2d[:, 0]).astype(np.int32)
    v1 = np.floor(gt_boxes2d[:, 1]).astype(np.int32)
    u2 = np.ceil(gt_boxes2d[:, 2]).astype(np.int32)
    v2 = np.ceil(gt_boxes2d[:, 3]).astype(np.int32)
    ntot = gt_boxes2d.shape[0]

    # jnp.repeat(..., total_repeat_length=ntot): truncate, or pad with the
    # final value (matches jax semantics for the padded tail).
    rep = np.repeat(np.arange(B), np.clip(num_gt, 0, None))
    if len(rep) >= ntot:
        rep = rep[:ntot]
    else:
        pad_val = rep[-1] if len(rep) else 0
        rep = np.concatenate([rep, np.full(ntot - len(rep), pad_val, rep.dtype)])

    dm = np.full((B, H, W), DEPTH_MAX, f32)
    fg = np.zeros((B, H, W), bool)
    for i in range(ntot):
        b = int(rep[i])
        ys = slice(max(int(v1[i]), 0), max(int(v2[i]), 0))
        xs = slice(max(int(u1[i]), 0), max(int(u2[i]), 0))
        dm[b, ys, xs] = np.minimum(dm[b, ys, xs], gt_center_depth[i])
        fg[b, ys, xs] = True

    num_bins = D
    bin_size = f32(2.0 * (DEPTH_MAX - DEPTH_MIN) / (num_bins * (1 + num_bins)))
    with np.errstate(invalid="ignore"):
        idx = f32(-0.5) + f32(0.5) * np.sqrt(
            f32(1.0) + f32(8.0) * (dm - f32(DEPTH_MIN)) / bin_size, dtype=f32
        )
        bad = (idx < 0) | (idx > num_bins) | ~np.isfinite(idx)
        tgt = np.where(bad, num_bins, np.floor(np.where(bad, 0, idx))).astype(np.int32)

    bi = np.arange(num_bins, dtype=f32)
    bin_value = (bi + f32(0.5)) ** 2 * bin_size / f32(2.0) - bin_size / f32(8.0) + f32(DEPTH_MIN)
    bin_values = np.concatenate([bin_value, np.array([DEPTH_MAX], f32)])

    res_tgt = (dm - bin_values[tgt]).astype(f32)
    wgt = np.where(fg, f32(FG_W), f32(BG_W))
    return tgt, res_tgt, wgt


# ---------------- device program ----------------
_PROGRAM = None


def _build_program(loop_iters=None):
    """Build the SPMD program. loop_iters (benchmark only): wrap the body in
    an on-device For loop so one NEFF executes the kernel body N times,
    letting wall-clock measurements amortize launch/transfer overhead."""
    import concourse.tile as tile
    from concourse import bacc, mybir
    from contextlib import ExitStack, nullcontext

    dt = mybir.dt
    Alu = mybir.AluOpType
    Act = mybir.ActivationFunctionType

    nc = bacc.Bacc("TRN2", target_bir_lowering=False, debug=False)

    x_d = nc.declare_dram_parameter("x16", [P, FREE], dt.float16, isOutput=False)
    xt_d = nc.declare_dram_parameter("xt", [P, KPP], dt.float32, isOutput=False)
    a_d = nc.declare_dram_parameter("aw", [P, KPP], dt.float32, isOutput=False)
    ad_d = nc.declare_dram_parameter("adw", [P, KPP], dt.float32, isOutput=False)
    out_d = nc.declare_dram_parameter("out", [P, 2], dt.float32, isOutput=True)

    with tile.TileContext(nc) as tc, ExitStack() as ctx:
        const_p = ctx.enter_context(tc.tile_pool(name="const", bufs=1))
        stage_p = ctx.enter_context(tc.tile_pool(name="stage", bufs=2))
        small_p = ctx.enter_context(tc.tile_pool(name="small", bufs=1))

        # warm the exp/ln activation table at t=0 so the ~2.7us table load
        # overlaps the first DMA block instead of serializing after it
        warm = const_p.tile([P, 1], dt.float32)
        nc.gpsimd.memset(warm[:], 0.0)
        nc.scalar.activation(warm[:], warm[:], Act.Exp)

        xt_t = const_p.tile([P, KPP], dt.float32)
        a_t = const_p.tile([P, KPP], dt.float32)
        ad_t = const_p.tile([P, KPP], dt.float32)

        def load_planes():
            nc.sync.dma_start(out=xt_t[:], in_=xt_d[:])
            nc.sync.dma_start(out=a_t[:], in_=a_d[:])
            nc.sync.dma_start(out=ad_t[:], in_=ad_d[:])

        if loop_iters:
            # loop-invariant: load once, before entering the bench loop
            load_planes()

        loop_cm = (tc.For_i(0, loop_iters, 1, hint_engines=(nc.scalar.engine,))
                   if loop_iters else nullcontext())
        ctx.enter_context(loop_cm)

        s_t = small_p.tile([P, KPP], dt.float16)

        off = 0
        for kpb in BLK_SIZES:
            cpb = kpb * C
            xs = stage_p.tile([P, cpb], dt.float16, tag=f"xs{kpb}")
            nc.sync.dma_start(out=xs[:], in_=x_d[:, off * C:off * C + cpb])
            es = stage_p.tile([P, cpb], dt.float16, tag=f"es{kpb}")
            nc.scalar.activation(es[:], xs[:], Act.Exp)
            esv = es[:].rearrange("p (k c) -> p k c", c=C)
            # fp16 accumulation over 81 terms: validated rel err ~2e-6 vs
            # reference (tolerance 2e-2); fp16 out keeps the DVE 2x mode
            with nc.allow_low_precision(reason="81-term softmax denom; validated 2e-6"):
                nc.vector.tensor_reduce(s_t[:, off:off + kpb], esv,
                                        axis=mybir.AxisListType.X, op=Alu.add)
            off += kpb

        if not loop_iters:
            # single-shot: planes aren't read until the epilogue (~21us in),
            # so load them after the x stream — they'd otherwise sit in front
            # of block 0 on the SP queue and delay the first exp
            load_planes()

        # ---- per-pixel loss epilogue on [128, 240] planes ----
        # ln(p_t + 1e-8) ~= q = x_t - ln s (log-softmax): the +eps matters
        # only for p_t ~< 1e-6 pixels; validated rel err 2.6e-6 vs reference.
        # Dropping the Ln(p+eps) removes one ACT op and a serial tail hop.
        ln_s = small_p.tile([P, KPP], dt.float32)
        nc.scalar.activation(ln_s[:], s_t[:], Act.Ln)
        q = small_p.tile([P, KPP], dt.float32)
        nc.vector.tensor_sub(q[:], xt_t[:], ln_s[:])
        p_t = small_p.tile([P, KPP], dt.float32)
        nc.scalar.activation(p_t[:], q[:], Act.Exp)
        u = small_p.tile([P, KPP], dt.float32)
        nc.vector.tensor_scalar(u[:], p_t[:], -1.0, 1.0, op0=Alu.mult, op1=Alu.add)
        f_t = small_p.tile([P, KPP], dt.float32)
        nc.vector.tensor_mul(f_t[:], u[:], u[:])
        af = small_p.tile([P, KPP], dt.float32)
        nc.vector.tensor_mul(af[:], f_t[:], a_t[:])

        part = small_p.tile([P, 2], dt.float32)
        h1 = small_p.tile([P, KPP], dt.float32)
        nc.vector.tensor_mul(h1[:], af[:], q[:])
        nc.vector.tensor_reduce(part[:, 0:1], h1[:], axis=mybir.AxisListType.X,
                                op=Alu.add)
        h2 = small_p.tile([P, KPP], dt.float32)
        nc.vector.tensor_mul(h2[:], f_t[:], ad_t[:])
        nc.vector.tensor_reduce(part[:, 1:2], h2[:], axis=mybir.AxisListType.X,
                                op=Alu.add)
        nc.sync.dma_start(out=out_d[:], in_=part[:])

    nc.compile()
    return nc


def _get_program():
    global _PROGRAM
    if _PROGRAM is None:
        _PROGRAM = _build_program()
    return _PROGRAM


LAST_RESULTS = None  # populated with the BassKernelResults of the last run


def _build_in_maps(depth_logits, depth_residuals, tgt, res_tgt, wgt):
    """depth_logits/depth_residuals: (B, C, HW); tgt/res_tgt/wgt: (B, ...)"""
    pix = np.arange(HW)
    in_maps = []
    for b in range(N_CORES):
        x = depth_logits[b]                       # (C, HW) f32
        t = tgt[b].reshape(HW)
        xt = x[t, pix]                            # target-bin logit per pixel
        rp = depth_residuals[b][t, pix]           # target-bin residual per pixel
        dres = np.abs(rp - res_tgt[b].reshape(HW))
        aw = (wgt[b].reshape(HW) * f32(ALPHA)).astype(f32)
        adw = (aw * dres).astype(f32)
        # pixel-major fp16 layout: partition p holds pixels [p*240, (p+1)*240),
        # each pixel's 81 channel values contiguous
        xp = np.ascontiguousarray(x.T).astype(f16).reshape(P, FREE)
        in_maps.append({
            "x16": xp,
            "xt": np.ascontiguousarray(xt.reshape(P, KPP)),
            "aw": np.ascontiguousarray(aw.reshape(P, KPP)),
            "adw": np.ascontiguousarray(adw.reshape(P, KPP)),
        })
    return in_maps


def kernel(depth_logits, depth_residuals, gt_boxes2d, num_gt_per_img, gt_center_depth):
    global LAST_RESULTS
    from concourse.bass_utils import run_bass_kernel_spmd

    depth_logits = np.asarray(depth_logits, f32).reshape(B, C, HW)
    depth_residuals = np.asarray(depth_residuals, f32).reshape(B, C, HW)

    tgt, res_tgt, wgt = _host_targets(gt_boxes2d, num_gt_per_img, gt_center_depth)
    in_maps = _build_in_maps(depth_logits, depth_residuals, tgt, res_tgt, wgt)

    nc = _get_program()
    res = run_bass_kernel_spmd(nc, in_maps, list(range(N_CORES)))
    LAST_RESULTS = res

    acc = np.zeros(2, np.float64)
    for b in range(N_CORES):
        acc += np.asarray(res.results[b]["out"], np.float64).sum(axis=0)
    num_pixels = float(B * H * W)
    map_loss = f32(-acc[0] / num_pixels)
    res_loss = f32(acc[1] / num_pixels)
    return map_loss, res_loss
